# revision 84
# baseline (speedup 1.0000x reference)
"""MoE routing kernel for 8 Trainium2 NeuronCores.

Strategy (expert-parallel, 3 launches; host does only data movement —
permutation/gather/pad/transpose/dtype-cast — between launches):
  L1 router : data-parallel over tokens. Logits = x16 @ gw with the
     gate-weight side residual-corrected in f16 (exact to ~1e-7); the x
     side is plain f16, which on this dataset flips the top-2 set for
     exactly 2 of 8192 tokens (both near-ties; 0.58% end-to-end error,
     measured, vs the 2e-2 gate) and keeps the DMA window to 2MB of x.
     gw16 is pre-scaled by 4096 (exact exponent shift) so both matmul
     groups per 128-token tile accumulate into ONE psum tile holding
     4096*logit: no DVE combine at all — DVE max/max_index read PSUM
     directly (max_index lands straight in the output tile) and the ACT
     sigmoid applies the 2^-12 descale via its input scale. x is staged
     per-chunk-contiguous ([p, (k t)] blocks, 2-4KB DMA elements).
  L2 experts : one expert per core. Host pre-gathers + transposes each
     expert's tokens into per-tile-contiguous fp16 blocks (one DMA per
     tile). Tile widths [256,256,256,512..] with phase order
     h1(0) h1(1) h1(2) h3(0) h3(1) h3(2) down(0) .. track the
     w1-then-w3-then-w2 weight stream so the PE never waits mid-launch;
     the first three tiles apply the routing gate on the OUTPUT side of
     the up/gate matmuls (h*gate on DVE) so no DVE work gates the first
     matmul. Gate also fused into the down-projection's PSUM->SBUF copy.
  L3 shared+combine : data-parallel over token slices, 256-token
     quarter phases pipelined the same way; the two routed
     contributions (host-permuted back from L2) are pre-summed on DVE
     and added in the down-projection's PSUM->SBUF copy.

fp16 for all GLU matmuls (fp8 e4m3 measures 2.6% norm error per
quantized tensor on this data — over the 2e-2 gate; hi/lo-corrected fp8
triples the matmul terms and loses to f16 outright at equal accuracy),
fp32 PSUM accumulation. L2/L3 front-load cheap f16 dummy matmuls (427ns
each at the mid p-state) so the PE ramp (0.65/1.2 -> 2.4GHz after ~3us
continuous busy) completes during the initial DMA wait. Phase widths
stay >= 256 tokens: narrower phases make the h1 m-block cadence outrun
the 0.73us/block w1 DMA stream and stall the PE.
"""
import sys
sys.path.insert(0, '/opt/trn_rl_repo')

import numpy as np

import concourse.bacc as bacc
import concourse.mybir as mybir
import concourse.tile as tile
from concourse.bass_utils import run_bass_kernel_spmd

F32 = mybir.dt.float32
F16 = mybir.dt.float16
F8 = mybir.dt.float8e4
U32 = mybir.dt.uint32
F8NP = mybir.dt.np(F8)
AF = mybir.ActivationFunctionType
ALU = mybir.AluOpType
DR = mybir.MatmulPerfMode.DoubleRow

NCORES = 8
E = 8           # experts
K = 2           # top-k
D = 1024
H = 1024
T = 8192        # total tokens (B*S)
TPC = T // NCORES   # tokens per core (router / combine slices)


def _warmup(nc, pin, pps, n=7, tag="wu", bufs=1):
    """Cheap f16 dummy matmuls (427ns each at the mid p-state): keep the PE
    continuously busy from t~1.3us so the ramp to 2.4GHz completes during
    the initial DMA wait. Output PSUM never read."""
    wu_in = pin.tile([128, 512], F16)
    nc.gpsimd.memset(wu_in[:], 0.0)
    wu_ps = pps.tile([128, 512], F32, tag=tag, bufs=bufs)
    for _ in range(n):
        nc.tensor.matmul(wu_ps[:], wu_in[:, 0:128], wu_in[:], start=True,
                         stop=True)


# --------------------------------------------------------------- L1: router
def build_l1(bias_vals):
    """Router on f16 x: psum = x16@(4096*gw16) + x16@rg4096 = 4096*(x16@gw)
    — the gate-weight side is residual-corrected (exact to ~1e-7) while the
    x side is f16. On this dataset that flips the top-2 set for exactly 2
    of 8192 tokens (both near-ties, 0.58%% end-to-end error, measured
    against the fp32 reference) and drops a 1MB fp8 x-residual load from
    the DMA critical path. The 4096 pre-scale makes both matmul groups
    accumulate into one PSUM tile, so there is no DVE combine on the
    critical path: just max/max_index (reading PSUM) and one
    scaled-sigmoid ACT op per 128-token tile."""
    bias_zero = all(float(b) == 0.0 for b in bias_vals)
    nc = bacc.Bacc("TRN2", target_bir_lowering=False, debug=False,
                   num_devices=NCORES)
    # packed [p, c, k, t]: value = x16[c*128 + t, k*128 + p]
    x16P = nc.dram_tensor("x16P", [128, 8, 8, 128], F16,
                          kind="ExternalInput").ap()
    # [0:32)f32 = gw16*4096, [32:64) = rg*4096 (f16)
    gwpk = nc.dram_tensor("gwpk", [128, 80], F32, kind="ExternalInput").ap()
    # packed output [p, tt, {gate0, gate1, idx8...}] (token = tt*128 + p);
    # max_index writes its full 8-wide result straight into [2:10] so no
    # extra copy sits on the critical chain
    go_o = nc.dram_tensor("go", [128, TPC // 128, 2 + 8], F32,
                          kind="ExternalOutput").ap()
    NT = TPC // 128

    with tile.TileContext(nc) as tc:
        with tc.tile_pool(name="pin", bufs=1) as pin, \
             tc.tile_pool(name="pps", bufs=8, space="PSUM") as pps, \
             tc.tile_pool(name="pwk", bufs=8) as pwk:
            xq = pin.tile([128, 8, 8, 128], F16)
            pk_sb = pin.tile([128, 80], F32)
            nc.sync.dma_start(xq[:, 0:2], x16P[:, 0:2])
            nc.sync.dma_start(pk_sb[:], gwpk[:])
            nc.sync.dma_start(xq[:, 2:4], x16P[:, 2:4])
            nc.sync.dma_start(xq[:, 4:6], x16P[:, 4:6])
            nc.sync.dma_start(xq[:, 6], x16P[:, 6])
            nc.sync.dma_start(xq[:, 7], x16P[:, 7])
            gw_k = [pk_sb[:, k*4:(k+1)*4].bitcast(F16) for k in range(8)]
            rg_k = [pk_sb[:, 32+k*4:32+(k+1)*4].bitcast(F16) for k in range(8)]
            goacc = pin.tile([128, NT, 2 + 8], F32)

            for tt in range(NT):
                xs_ = xq[:, tt]
                ps = pps.tile([128, E], F32, tag="ps")
                for k in range(8):
                    nc.tensor.matmul(ps[:], xs_[:, k], gw_k[k],
                                     start=(k == 0), stop=False)
                for k in range(8):
                    nc.tensor.matmul(ps[:], xs_[:, k], rg_k[k],
                                     start=False, stop=(k == 7))
                if bias_zero:
                    # selection key = psum (sigmoid monotone, bias 0);
                    # true gates = sigmoid(psum/4096) fused into the ACT op,
                    # issued right after max so it overlaps max_index (DVE)
                    top8 = pwk.tile([128, 8], F32, tag="top8")
                    nc.vector.max(top8[:], ps[:])
                    nc.scalar.activation(goacc[:, tt, 0:K], top8[:, 0:K],
                                         AF.Sigmoid, scale=1.0 / 4096.0)
                    nc.vector.max_index(goacc[:, tt, 2:10].bitcast(U32),
                                        top8[:], ps[:])
                else:
                    # general path: key = sigmoid(logit) + bias[e]; true
                    # gate = key[selected] - bias[selected]
                    sel = pwk.tile([128, E], F32, tag="sel")
                    nc.scalar.activation(sel[:], ps[:], AF.Sigmoid,
                                         scale=1.0 / 4096.0)
                    for e in range(E):
                        if float(bias_vals[e]) != 0.0:
                            nc.vector.tensor_scalar_add(sel[:, e:e+1],
                                                        sel[:, e:e+1],
                                                        float(bias_vals[e]))
                    top8 = pwk.tile([128, 8], F32, tag="top8")
                    nc.vector.max(top8[:], sel[:])
                    nc.vector.max_index(goacc[:, tt, 2:10].bitcast(U32),
                                        top8[:], sel[:])
                    idxf = pwk.tile([128, K], F32, tag="idxf")
                    nc.vector.tensor_copy(idxf[:],
                                          goacc[:, tt, 2:2+K].bitcast(U32))
                    gates = pwk.tile([128, K], F32, tag="gts")
                    nc.vector.tensor_copy(gates[:], top8[:, 0:K])
                    for e in range(E):
                        if float(bias_vals[e]) == 0.0:
                            continue
                        msk = pwk.tile([128, K], F32, tag="msk")
                        nc.vector.tensor_scalar(msk[:], idxf[:], float(e),
                                                None, op0=ALU.is_equal)
                        nc.vector.tensor_scalar_mul(msk[:], msk[:],
                                                    -float(bias_vals[e]))
                        nc.vector.tensor_add(gates[:], gates[:], msk[:])
                    nc.vector.tensor_copy(goacc[:, tt, 0:K], gates[:])
                if tt == NT // 2 - 1:
                    nc.sync.dma_start(go_o[:, 0:NT//2], goacc[:, 0:NT//2])
            nc.sync.dma_start(go_o[:, NT//2:NT], goacc[:, NT//2:NT])
    nc.compile()
    return nc


# -------------------------------------------------------------- B: experts
def _b_widths(cape):
    widths = []
    rem = cape
    for w in (256, 256, 256):
        if rem <= 0:
            break
        w = min(w, rem)
        widths.append(w)
        rem -= w
    while rem > 768:
        widths.append(512)
        rem -= 512
    if rem > 512:
        a = -(-rem // 64) * 32
        widths += [a, rem - a]
    elif rem:
        widths.append(rem)
    return widths


def build_B(cape):
    assert cape % 2 == 0
    widths = _b_widths(cape)
    ntiles = len(widths)
    starts = [sum(widths[:i]) for i in range(ntiles)]
    ngated = min(3, ntiles)     # leading tiles use output-side gating

    nc = bacc.Bacc("TRN2", target_bir_lowering=False, debug=False,
                   num_devices=NCORES)
    # per-tile contiguous blocks: tile i at [128, 8*starts[i] : +8*widths[i]],
    # inside a block value (p, k, t) = x16[token starts[i]+t, k*128+p]
    # f16 x is pre-scaled *16 and f16 w1/w3 *256 on the host (exact
    # exponent shifts) so the f16 and fp8-DoubleRow matmul terms share one
    # 4096x PSUM scale; gt12 = gate*2^-12 folds the descale into the
    # output-side gate multiply before silu / the glu mul
    xeP = nc.dram_tensor("xeP", [128, 8 * cape], F16, kind="ExternalInput").ap()
    gatb = nc.dram_tensor("gatb", [128, cape], F16, kind="ExternalInput").ap()
    gt12b = nc.dram_tensor("gt12b", [128, cape], F16, kind="ExternalInput").ap()
    w1M = nc.dram_tensor("w1M", [8, 128, 8, 128], F16, kind="ExternalInput").ap()
    w1F8 = nc.dram_tensor("w1F8", [128, 2, H], F8, kind="ExternalInput").ap()
    w3T = nc.dram_tensor("w3T", [D, H], F16, kind="ExternalInput").ap()
    w2T = nc.dram_tensor("w2T", [H, D], F16, kind="ExternalInput").ap()
    yT_o = nc.dram_tensor("yT", [D, cape], F16, kind="ExternalOutput").ap()

    with tile.TileContext(nc) as tc:
        with tc.tile_pool(name="pin", bufs=1) as pin, \
             tc.tile_pool(name="pps", bufs=2, space="PSUM") as pps, \
             tc.tile_pool(name="px", bufs=4) as px, \
             tc.tile_pool(name="px8", bufs=4) as px8, \
             tc.tile_pool(name="pg", bufs=4) as pg, \
             tc.tile_pool(name="pmg", bufs=3) as pmg, \
             tc.tile_pool(name="ps1", bufs=4) as ps1, \
             tc.tile_pool(name="po", bufs=2) as po:
            _warmup(nc, pin, pps, n=7, tag="y", bufs=2)
            gat_sb = pin.tile([128, cape], F16)
            gt12_sb = pin.tile([128, cape], F16)
            w1r = pin.tile([128, 8, 8, 128], F16)   # [p, m, k, c]
            w1q = pin.tile([128, 2, H], F8)         # [p, i, h] fp8 d[0:256]
            w3r = pin.tile([128, 8, H], F16)
            w2r = pin.tile([128, 8, D], F16)

            def load_xs_raw(t):
                tw = widths[t]
                t0 = starts[t]
                xs = px.tile([128, 8, 512], F16, tag="xs")
                src = xeP[:, 8*t0:8*(t0+tw)].rearrange("p (k t) -> p k t", k=8)
                nc.sync.dma_start(xs[:, :, 0:tw], src)
                # device-side fp8 copy of d[0:256] for the DoubleRow part
                # (x already carries the *16 scale)
                x8 = px8.tile([128, 2, 512], F8, tag="x8")
                nc.vector.tensor_copy(x8[:, :, 0:tw], xs[:, 0:2, 0:tw])
                return xs, x8

            # startup stream: w1 m-block 0, fp8 w1 slice, tile-0 x, gate
            # slices, w1 m-blocks 1-7, x tiles 1-2, big weights
            nc.sync.dma_start(w1r[:, 0, :, :], w1M[0])
            nc.sync.dma_start(w1q[:], w1F8[:])
            xs_t = [None] * ntiles
            x8_t = [None] * ntiles
            xs_t[0], x8_t[0] = load_xs_raw(0)
            g01 = sum(widths[0:ngated])
            nc.sync.dma_start(gt12_sb[:, 0:g01], gt12b[:, 0:g01])
            for m in range(1, 8):
                nc.sync.dma_start(w1r[:, m, :, :], w1M[m])
            for t in range(1, ngated):
                xs_t[t], x8_t[t] = load_xs_raw(t)
            nc.sync.dma_start(w3r[:], w3T.rearrange("(k p) h -> p k h", p=128))
            if ntiles > ngated:
                xs_t[ngated], x8_t[ngated] = load_xs_raw(ngated)
            nc.sync.dma_start(gat_sb[:], gatb[:])
            nc.sync.dma_start(w2r[:], w2T.rearrange("(m p) d -> p m d", p=128))
            if ntiles > ngated:
                nc.sync.dma_start(gt12_sb[:, g01:cape], gt12b[:, g01:cape])

            def h1_phase(t):
                xs = xs_t[t]
                x8 = x8_t[t]
                tw = widths[t]
                t0 = starts[t]
                nhs = [(0, min(256, tw))] + ([(256, tw)] if tw > 256 else [])
                s1a = ps1.tile([128, 8, 512], F16, tag="s1a")
                for m in range(8):
                    h1 = pps.tile([128, 512], F32, tag="h1")
                    # f16 k=2 opens the psum group over the full [128, tw]
                    # zero-region; f16 k=7 closes it
                    nc.tensor.matmul(h1[:, 0:tw], w1r[:, m, 2, :],
                                     xs[:, 2, 0:tw], start=True, stop=False)
                    # d[0:256] in fp8 DoubleRow. Lower 64 output rows
                    # accumulate straight into h1 (mixed group, base 0);
                    # the backend rejects DR at base 64, so the upper rows
                    # land in their own [64, n] psum tiles and merge below.
                    h8s = []
                    for (n0, n1) in nhs:
                        nc.tensor.matmul(h1[0:64, n0:n1],
                                         w1q[:, :, m*128:m*128+64],
                                         x8[:, :, n0:n1],
                                         start=False, stop=False, perf_mode=DR)
                        h8 = pps.tile([64, 256], F32, tag="h8", bufs=2)
                        nc.tensor.matmul(h8[:, 0:n1-n0],
                                         w1q[:, :, m*128+64:m*128+128],
                                         x8[:, :, n0:n1],
                                         start=True, stop=True, perf_mode=DR)
                        h8s.append((h8, n0, n1))
                    for k in range(3, 8):
                        nc.tensor.matmul(h1[:, 0:tw], w1r[:, m, k, :],
                                         xs[:, k, 0:tw],
                                         start=False, stop=(k == 7))
                    # descale the upper-row fp8 parts early (overlaps the
                    # f16 matmuls), then gate-descale h1 and merge with a
                    # cross-partition add
                    # the descale mul carries the partition shift (out at
                    # base 64, both inputs at base 0 — a 2-input-base op,
                    # which walrus accepts; a 3-base add does not)
                    tmps = []
                    for (h8, n0, n1) in h8s:
                        tmp = pmg.tile([128, 256], F16, tag="tmp")
                        nc.vector.tensor_mul(tmp[64:128, 0:n1-n0],
                                             h8[:, 0:n1-n0],
                                             gt12_sb[0:64, t0+n0:t0+n1])
                        tmps.append((tmp, n0, n1))
                    m1 = pmg.tile([128, 512], F16, tag="m1")
                    nc.vector.tensor_mul(m1[:, 0:tw], h1[:, 0:tw],
                                         gt12_sb[:, t0:t0+tw])
                    for (tmp, n0, n1) in tmps:
                        nc.vector.tensor_add(m1[64:128, n0:n1],
                                             m1[64:128, n0:n1],
                                             tmp[64:128, 0:n1-n0])
                    nc.scalar.activation(s1a[:, m, 0:tw], m1[:, 0:tw],
                                         AF.Silu)
                return s1a

            def h3_phase(t, s1a):
                xs = xs_t[t]
                tw = widths[t]
                t0 = starts[t]
                gT = pg.tile([128, 8, 512], F16, tag="gT")
                for m in range(8):
                    h3 = pps.tile([128, 512], F32, tag="h3")
                    for k in range(8):
                        nc.tensor.matmul(h3[:, 0:tw], w3r[:, k, m*128:(m+1)*128],
                                         xs[:, k, 0:tw],
                                         start=(k == 0), stop=(k == 7))
                    m3 = pmg.tile([128, 512], F16, tag="m3")
                    nc.vector.tensor_mul(m3[:, 0:tw], h3[:, 0:tw],
                                         gt12_sb[:, t0:t0+tw])
                    nc.vector.tensor_mul(gT[:, m, 0:tw], s1a[:, m, 0:tw],
                                         m3[:, 0:tw])
                return gT

            def down(gT, t):
                tw = widths[t]
                t0 = starts[t]
                last = (t == ntiles - 1)
                osb = po.tile([128, 8, 512], F16, tag="osb")
                yTr = yT_o.rearrange("(d p) t -> p d t", p=128)
                for d in range(8):
                    yp = pps.tile([128, 512], F32, tag="y")
                    for m in range(8):
                        nc.tensor.matmul(yp[:, 0:tw], w2r[:, m, d*128:(d+1)*128],
                                         gT[:, m, 0:tw],
                                         start=(m == 0), stop=(m == 7))
                    # PSUM->SBUF copy fused with the output-side routing scale
                    nc.vector.tensor_mul(osb[:, d, 0:tw], yp[:, 0:tw],
                                         gat_sb[:, t0:t0+tw])
                    if last:
                        nc.sync.dma_start(yTr[:, d, t0:t0+tw], osb[:, d, 0:tw])
                if not last:
                    nc.gpsimd.dma_start(yTr[:, :, t0:t0+tw], osb[:, :, 0:tw])

            # pipeline: ngated h1 phases up front (w1-only work) so the PE
            # has runway while w3 streams; then rotate h1/h3/down
            s1a_t = [None] * ntiles
            gT_t = [None] * ntiles
            for t in range(ngated):
                s1a_t[t] = h1_phase(t)
            for t in range(ngated):
                gT_t[t] = h3_phase(t, s1a_t[t])
            if ntiles > ngated:
                for j in range(max(1, ngated - 2)):
                    down(gT_t[j], j)
                for t in range(ngated, ntiles):
                    if xs_t[t] is None:
                        xs_t[t], x8_t[t] = load_xs_raw(t)
                    s1a_t[t] = h1_phase(t)
                    gT_t[t] = h3_phase(t, s1a_t[t])
                    down(gT_t[t - 2], t - 2)
                down(gT_t[ntiles - 2], ntiles - 2)
                down(gT_t[ntiles - 1], ntiles - 1)
            else:
                for t in range(ntiles):
                    down(gT_t[t], t)
    nc.compile()
    return nc


# ------------------------------------------------------ L3: shared + combine
def build_l3():
    nc = bacc.Bacc("TRN2", target_bir_lowering=False, debug=False,
                   num_devices=NCORES)
    xP = nc.dram_tensor("xP", [128, 8, 8, 128], F16,
                        kind="ExternalInput").ap()
    sw1B = nc.dram_tensor("sw1B", [8, 128, 8, 128], F16,
                          kind="ExternalInput").ap()
    sw3B = nc.dram_tensor("sw3B", [D, H], F16, kind="ExternalInput").ap()
    sw2B = nc.dram_tensor("sw2B", [H, D], F16, kind="ExternalInput").ap()
    AT = nc.dram_tensor("AT", [D, TPC], F16, kind="ExternalInput").ap()
    BT = nc.dram_tensor("BT", [D, TPC], F16, kind="ExternalInput").ap()
    out_o = nc.dram_tensor("outT", [D, TPC], F16, kind="ExternalOutput").ap()
    # phases over 128-token chunks: 4 quarters of 256 (phase width below
    # 256 makes the h1 m-cadence outrun the w1 DMA stream and stalls)
    CH = [(0, 2), (2, 4), (4, 6), (6, 8)]
    NM = 8

    with tile.TileContext(nc) as tc:
        with tc.tile_pool(name="pin", bufs=1) as pin, \
             tc.tile_pool(name="pps", bufs=2, space="PSUM") as pps, \
             tc.tile_pool(name="pg", bufs=3) as pg, \
             tc.tile_pool(name="ps1", bufs=3) as ps1, \
             tc.tile_pool(name="po", bufs=2) as po:
            _warmup(nc, pin, pps, n=7)
            xs = pin.tile([128, 8, 8, 128], F16)
            w1r = pin.tile([128, 8, 8, 128], F16)
            w3r = pin.tile([128, 8, H], F16)
            w2r = pin.tile([128, 8, D], F16)
            absum = pin.tile([128, 8, TPC], F16)
            bt_sb = pin.tile([128, 8, TPC], F16)
            ATr = AT.rearrange("(d p) t -> p d t", p=128)
            BTr = BT.rearrange("(d p) t -> p d t", p=128)

            nc.sync.dma_start(w1r[:, 0], sw1B[0])
            nc.sync.dma_start(xs[:, 0], xP[:, 0])
            nc.sync.dma_start(xs[:, 1], xP[:, 1])
            for m in range(1, 8):
                nc.sync.dma_start(w1r[:, m], sw1B[m])
            nc.sync.dma_start(xs[:, 2:4], xP[:, 2:4])
            nc.sync.dma_start(w3r[:], sw3B.rearrange("(k p) h -> p k h", p=128))
            nc.sync.dma_start(xs[:, 4:6], xP[:, 4:6])
            nc.sync.dma_start(xs[:, 6:8], xP[:, 6:8])
            # combine terms: first halves, then w2, then the rest
            nc.sync.dma_start(absum[:, :, 0:512], ATr[:, :, 0:512])
            nc.sync.dma_start(bt_sb[:, :, 0:512], BTr[:, :, 0:512])
            nc.sync.dma_start(w2r[:], sw2B.rearrange("(m p) d -> p m d", p=128))
            nc.sync.dma_start(absum[:, :, 512:1024], ATr[:, :, 512:1024])
            nc.sync.dma_start(bt_sb[:, :, 512:1024], BTr[:, :, 512:1024])

            def absum_add(half):
                ts = slice(half*512, (half+1)*512)
                nc.vector.tensor_add(absum[:, :, ts], absum[:, :, ts],
                                     bt_sb[:, :, ts])

            def h1_phase(q):
                c0, c1 = CH[q]
                w = (c1 - c0) * 128
                s1a = ps1.tile([128, NM, 256], F16, tag="s1a")
                for m in range(NM):
                    h1 = pps.tile([128, 256], F32, tag="h1")
                    for k in range(8):
                        nc.tensor.matmul(h1[:, 0:w], w1r[:, m, k, :],
                                         xs[:, c0:c1, k, :],
                                         start=(k == 0), stop=(k == 7))
                    nc.scalar.activation(s1a[:, m, 0:w], h1[:, 0:w], AF.Silu)
                return s1a

            def h3_phase(q, s1a):
                c0, c1 = CH[q]
                w = (c1 - c0) * 128
                gT = pg.tile([128, NM, 256], F16, tag="gT")
                for m in range(NM):
                    h3 = pps.tile([128, 256], F32, tag="h3")
                    for k in range(8):
                        nc.tensor.matmul(h3[:, 0:w], w3r[:, k, m*128:(m+1)*128],
                                         xs[:, c0:c1, k, :],
                                         start=(k == 0), stop=(k == 7))
                    nc.vector.tensor_mul(gT[:, m, 0:w], s1a[:, m, 0:w],
                                         h3[:, 0:w])
                return gT

            def down(gT, q):
                c0, c1 = CH[q]
                w = (c1 - c0) * 128
                t0 = c0 * 128
                osb = po.tile([128, 8, 256], F16, tag="osb")
                outr = out_o.rearrange("(d p) t -> p d t", p=128)
                for d in range(8):
                    yp = pps.tile([128, 256], F32, tag="y")
                    for m in range(NM):
                        nc.tensor.matmul(yp[:, 0:w], w2r[:, m, d*128:(d+1)*128],
                                         gT[:, m, 0:w],
                                         start=(m == 0), stop=(m == NM-1))
                    nc.vector.tensor_add(osb[:, d, 0:w], yp[:, 0:w],
                                         absum[:, d, t0:t0+w])
                    nc.sync.dma_start(outr[:, d, t0:t0+w], osb[:, d, 0:w])

            s0 = h1_phase(0)
            s1 = h1_phase(1)
            g0 = h3_phase(0, s0)
            absum_add(0)
            s2 = h1_phase(2)
            g1 = h3_phase(1, s1)
            down(g0, 0)
            s3 = h1_phase(3)
            absum_add(1)
            g2 = h3_phase(2, s2)
            down(g1, 1)
            g3 = h3_phase(3, s3)
            down(g2, 2)
            down(g3, 3)
    nc.compile()
    return nc


_BUILT = {}
_LAST_KEYS = []


def _get(name, builder, *args):
    key = (name,) + tuple(args)
    if key not in _BUILT:
        _BUILT[key] = builder(*args)
    return _BUILT[key], key


def _packx(a, chunk):
    """[Tn, D] -> [128, Tn//chunk, 8, chunk]: out[p, c, k, t] =
    a[c*chunk + t, k*128 + p] (per-chunk-contiguous DMA layout)."""
    n = a.shape[0] // chunk
    return np.ascontiguousarray(
        a.reshape(n, chunk, 8, 128).transpose(3, 0, 2, 1))


def _mblocks(wT16, nm=8):
    # [D, nm*128] -> [m, p, k, c] with [m,p,k,c] = wT[k*128+p, m*128+c]
    return np.ascontiguousarray(
        wT16.reshape(8, 128, nm, 128).transpose(2, 1, 0, 3))


def kernel(**inputs):
    x = np.ascontiguousarray(np.asarray(inputs["x"], dtype=np.float32))
    xf = x.reshape(T, D)
    gw = np.asarray(inputs["gate_w"], dtype=np.float32)
    bias = np.asarray(inputs["expert_bias"], dtype=np.float32)
    w1 = np.asarray(inputs["w1"], dtype=np.float32)
    w2 = np.asarray(inputs["w2"], dtype=np.float32)
    w3 = np.asarray(inputs["w3"], dtype=np.float32)
    sw1 = np.asarray(inputs["sw1"], dtype=np.float32)
    sw2 = np.asarray(inputs["sw2"], dtype=np.float32)
    sw3 = np.asarray(inputs["sw3"], dtype=np.float32)

    cores = list(range(NCORES))
    del _LAST_KEYS[:]

    # ---- L1: router ----
    ncA, kA = _get("l1", build_l1, tuple(float(b) for b in bias))
    _LAST_KEYS.append(kA)
    xf16 = xf.astype(np.float16)
    gw16T = np.ascontiguousarray(gw.astype(np.float16).T)
    rgT = np.ascontiguousarray(
        ((gw.T - gw16T.astype(np.float32)) * 4096.0).astype(np.float16))

    def _prow(a):
        # [D, E] -> per-partition [128, 8*E] rows matching the [p][k][e] reads
        return np.ascontiguousarray(
            a.reshape(8, 128, E).transpose(1, 0, 2)).reshape(128, 8 * E)

    gw16s = (gw16T.astype(np.float32) * 4096.0).astype(np.float16)
    pk = np.zeros((128, 320), np.uint8)
    pk[:, 0:128] = _prow(gw16s).view(np.uint8)
    pk[:, 128:256] = _prow(rgT).view(np.uint8)
    gwpk = np.ascontiguousarray(pk).view(np.float32)
    xPl = [_packx(xf16[c*TPC:(c+1)*TPC], 128) for c in cores]
    inA = [{"x16P": xPl[c], "gwpk": gwpk} for c in cores]
    rA = run_bass_kernel_spmd(ncA, inA, cores).results
    gates_l, sel_l = [], []
    for r in rA:
        go = np.ascontiguousarray(r["go"])     # [128, NT, 10]
        gates_l.append(go[:, :, 0:2].transpose(1, 0, 2).reshape(TPC, K))
        sel_l.append(
            go.view(np.uint32)[:, :, 2:4].transpose(1, 0, 2).reshape(TPC, K))
    gates = np.concatenate(gates_l)
    sel = np.concatenate(sel_l)

    # ---- host dispatch (pure data movement: stable sort by expert) ----
    flat = sel.reshape(-1).astype(np.int64)
    order = np.argsort(flat, kind="stable")               # [T*K]
    toks = order // K
    kslot = order % K
    gs = gates.reshape(-1)[order]
    counts = np.bincount(flat, minlength=E)
    bounds = np.concatenate([[0], np.cumsum(counts)])
    cape = int(-(-int(counts.max()) // 2) * 2)

    # ---- B: experts ----
    ncB, kB = _get("B", build_B, cape)
    _LAST_KEYS.append(kB)
    widths = _b_widths(cape)
    starts = [sum(widths[:i]) for i in range(len(widths))]
    inB = []
    for e in cores:
        n = int(counts[e])
        sl = slice(int(bounds[e]), int(bounds[e+1]))
        # f16 x pre-scaled *16 so the f16 and fp8 matmul terms share one
        # 4096x psum scale (w1/w3 carry *256)
        xe = np.zeros((cape, D), np.float16)
        xe[:n] = (xf16[toks[sl]].astype(np.float32) * 16.0).astype(np.float16)
        xeP = np.concatenate(
            [_packx(xe[t0:t0+tw], tw).reshape(128, 8*tw)
             for t0, tw in zip(starts, widths)], axis=1)
        gb = np.zeros((128, cape), np.float16)
        gb[:, :n] = gs[sl].astype(np.float16)[None, :]
        gt12 = (gb.astype(np.float32) * (1.0 / 4096.0)).astype(np.float16)
        w1t = w1[e].T.astype(np.float32) * 256.0  # [D, H]
        w3t = w3[e].T.astype(np.float32) * 256.0
        # fp8 stationary [p, i, h] = w1[h, i*128+p]*256
        w1f8 = np.ascontiguousarray(
            w1t[0:256].reshape(2, 128, H).transpose(1, 0, 2)).astype(F8NP)
        inB.append({
            "xeP": np.ascontiguousarray(xeP),
            "gatb": gb,
            "gt12b": gt12,
            "w1M": _mblocks(w1t.astype(np.float16)),
            "w1F8": w1f8,
            "w3T": np.ascontiguousarray(w3t.astype(np.float16)),
            "w2T": np.ascontiguousarray(w2[e].T).astype(np.float16),
        })
    rB = run_bass_kernel_spmd(ncB, inB, cores).results

    # ---- host combine prep (pure data movement: permutation) ----
    A = np.zeros((T, D), np.float16)
    B = np.zeros((T, D), np.float16)
    for e in cores:
        n = int(counts[e])
        sl = slice(int(bounds[e]), int(bounds[e+1]))
        rows = rB[e]["yT"][:, :n].T                       # [n, D] f16
        tsel = toks[sl]
        ksel = kslot[sl]
        A[tsel[ksel == 0]] = rows[ksel == 0]
        B[tsel[ksel == 1]] = rows[ksel == 1]

    # ---- L3: shared + combine ----
    ncC, kC = _get("l3", build_l3)
    _LAST_KEYS.append(kC)
    sw1B = _mblocks(sw1.T.astype(np.float16))
    sw3B = np.ascontiguousarray(sw3.T).astype(np.float16)
    sw2B = np.ascontiguousarray(sw2.T).astype(np.float16)
    inC = []
    for c in cores:
        sl = slice(c*TPC, (c+1)*TPC)
        inC.append({
            "xP": xPl[c],
            "sw1B": sw1B, "sw3B": sw3B, "sw2B": sw2B,
            "AT": np.ascontiguousarray(A[sl].T),
            "BT": np.ascontiguousarray(B[sl].T),
        })
    rC = run_bass_kernel_spmd(ncC, inC, cores).results
    out = np.concatenate([r["outT"].astype(np.float32).T for r in rC])
    return out.reshape(x.shape).astype(inputs["x"].dtype, copy=False)


# revision 85
# speedup vs baseline: 1.0002x; 1.0002x over previous
"""MoE routing kernel for 8 Trainium2 NeuronCores.

Strategy (expert-parallel, 3 launches; host does only data movement —
permutation/gather/pad/transpose/dtype-cast — between launches):
  L1 router : data-parallel over tokens. Logits = x16 @ gw with the
     gate-weight side residual-corrected in f16 (exact to ~1e-7); the x
     side is plain f16, which on this dataset flips the top-2 set for
     exactly 2 of 8192 tokens (both near-ties; 0.58% end-to-end error,
     measured, vs the 2e-2 gate) and keeps the DMA window to 2MB of x.
     gw16 is pre-scaled by 4096 (exact exponent shift) so both matmul
     groups per 128-token tile accumulate into ONE psum tile holding
     4096*logit: no DVE combine at all — DVE max/max_index read PSUM
     directly (max_index lands straight in the output tile) and the ACT
     sigmoid applies the 2^-12 descale via its input scale. x is staged
     per-chunk-contiguous ([p, (k t)] blocks, 2-4KB DMA elements).
  L2 experts : one expert per core. Host pre-gathers + transposes each
     expert's tokens into per-tile-contiguous fp16 blocks (one DMA per
     tile). Tile widths [256,256,256,512..] with phase order
     h1(0) h1(1) h1(2) h3(0) h3(1) h3(2) down(0) .. track the
     w1-then-w3-then-w2 weight stream so the PE never waits mid-launch;
     the first three tiles apply the routing gate on the OUTPUT side of
     the up/gate matmuls (h*gate on DVE) so no DVE work gates the first
     matmul. Gate also fused into the down-projection's PSUM->SBUF copy.
  L3 shared+combine : data-parallel over token slices, 256-token
     quarter phases pipelined the same way; the two routed
     contributions (host-permuted back from L2) are pre-summed on DVE
     and added in the down-projection's PSUM->SBUF copy.

fp16 for all GLU matmuls (fp8 e4m3 measures 2.6% norm error per
quantized tensor on this data — over the 2e-2 gate; hi/lo-corrected fp8
triples the matmul terms and loses to f16 outright at equal accuracy),
fp32 PSUM accumulation. L2/L3 front-load cheap f16 dummy matmuls (427ns
each at the mid p-state) so the PE ramp (0.65/1.2 -> 2.4GHz after ~3us
continuous busy) completes during the initial DMA wait. Phase widths
stay >= 256 tokens: narrower phases make the h1 m-block cadence outrun
the 0.73us/block w1 DMA stream and stall the PE.
"""
import sys
sys.path.insert(0, '/opt/trn_rl_repo')

import numpy as np

import concourse.bacc as bacc
import concourse.mybir as mybir
import concourse.tile as tile
from concourse.bass_utils import run_bass_kernel_spmd

F32 = mybir.dt.float32
F16 = mybir.dt.float16
F8 = mybir.dt.float8e4
U32 = mybir.dt.uint32
F8NP = mybir.dt.np(F8)
AF = mybir.ActivationFunctionType
ALU = mybir.AluOpType
DR = mybir.MatmulPerfMode.DoubleRow

NCORES = 8
E = 8           # experts
K = 2           # top-k
D = 1024
H = 1024
T = 8192        # total tokens (B*S)
TPC = T // NCORES   # tokens per core (router / combine slices)


def _warmup(nc, pin, pps, n=7, tag="wu", bufs=1):
    """Cheap f16 dummy matmuls (427ns each at the mid p-state): keep the PE
    continuously busy from t~1.3us so the ramp to 2.4GHz completes during
    the initial DMA wait. Output PSUM never read."""
    wu_in = pin.tile([128, 512], F16)
    nc.gpsimd.memset(wu_in[:], 0.0)
    wu_ps = pps.tile([128, 512], F32, tag=tag, bufs=bufs)
    for _ in range(n):
        nc.tensor.matmul(wu_ps[:], wu_in[:, 0:128], wu_in[:], start=True,
                         stop=True)


# --------------------------------------------------------------- L1: router
def build_l1(bias_vals):
    """Router on f16 x: psum = x16@(4096*gw16) + x16@rg4096 = 4096*(x16@gw)
    — the gate-weight side is residual-corrected (exact to ~1e-7) while the
    x side is f16. On this dataset that flips the top-2 set for exactly 2
    of 8192 tokens (both near-ties, 0.58%% end-to-end error, measured
    against the fp32 reference) and drops a 1MB fp8 x-residual load from
    the DMA critical path. The 4096 pre-scale makes both matmul groups
    accumulate into one PSUM tile, so there is no DVE combine on the
    critical path: just max/max_index (reading PSUM) and one
    scaled-sigmoid ACT op per 128-token tile."""
    bias_zero = all(float(b) == 0.0 for b in bias_vals)
    nc = bacc.Bacc("TRN2", target_bir_lowering=False, debug=False,
                   num_devices=NCORES)
    # packed [p, c, k, t]: value = x16[c*128 + t, k*128 + p]
    x16P = nc.dram_tensor("x16P", [128, 8, 8, 128], F16,
                          kind="ExternalInput").ap()
    # [0:32)f32 = gw16*4096, [32:64) = rg*4096 (f16)
    gwpk = nc.dram_tensor("gwpk", [128, 80], F32, kind="ExternalInput").ap()
    # packed output [p, tt, {gate0, gate1, idx8...}] (token = tt*128 + p);
    # max_index writes its full 8-wide result straight into [2:10] so no
    # extra copy sits on the critical chain
    go_o = nc.dram_tensor("go", [128, TPC // 128, 2 + 8], F32,
                          kind="ExternalOutput").ap()
    NT = TPC // 128

    with tile.TileContext(nc) as tc:
        with tc.tile_pool(name="pin", bufs=1) as pin, \
             tc.tile_pool(name="pps", bufs=8, space="PSUM") as pps, \
             tc.tile_pool(name="pwk", bufs=8) as pwk:
            xq = pin.tile([128, 8, 8, 128], F16)
            pk_sb = pin.tile([128, 80], F32)
            nc.sync.dma_start(xq[:, 0:2], x16P[:, 0:2])
            nc.sync.dma_start(pk_sb[:], gwpk[:])
            nc.sync.dma_start(xq[:, 2:4], x16P[:, 2:4])
            nc.sync.dma_start(xq[:, 4:6], x16P[:, 4:6])
            nc.sync.dma_start(xq[:, 6], x16P[:, 6])
            nc.sync.dma_start(xq[:, 7], x16P[:, 7])
            gw_k = [pk_sb[:, k*4:(k+1)*4].bitcast(F16) for k in range(8)]
            rg_k = [pk_sb[:, 32+k*4:32+(k+1)*4].bitcast(F16) for k in range(8)]
            goacc = pin.tile([128, NT, 2 + 8], F32)

            for tt in range(NT):
                xs_ = xq[:, tt]
                ps = pps.tile([128, E], F32, tag="ps")
                for k in range(8):
                    nc.tensor.matmul(ps[:], xs_[:, k], gw_k[k],
                                     start=(k == 0), stop=False)
                for k in range(8):
                    nc.tensor.matmul(ps[:], xs_[:, k], rg_k[k],
                                     start=False, stop=(k == 7))
                if bias_zero:
                    # selection key = psum (sigmoid monotone, bias 0);
                    # true gates = sigmoid(psum/4096) fused into the ACT op,
                    # issued right after max so it overlaps max_index (DVE)
                    top8 = pwk.tile([128, 8], F32, tag="top8")
                    nc.vector.max(top8[:], ps[:])
                    nc.scalar.activation(goacc[:, tt, 0:K], top8[:, 0:K],
                                         AF.Sigmoid, scale=1.0 / 4096.0)
                    nc.vector.max_index(goacc[:, tt, 2:10].bitcast(U32),
                                        top8[:], ps[:])
                else:
                    # general path: key = sigmoid(logit) + bias[e]; true
                    # gate = key[selected] - bias[selected]
                    sel = pwk.tile([128, E], F32, tag="sel")
                    nc.scalar.activation(sel[:], ps[:], AF.Sigmoid,
                                         scale=1.0 / 4096.0)
                    for e in range(E):
                        if float(bias_vals[e]) != 0.0:
                            nc.vector.tensor_scalar_add(sel[:, e:e+1],
                                                        sel[:, e:e+1],
                                                        float(bias_vals[e]))
                    top8 = pwk.tile([128, 8], F32, tag="top8")
                    nc.vector.max(top8[:], sel[:])
                    nc.vector.max_index(goacc[:, tt, 2:10].bitcast(U32),
                                        top8[:], sel[:])
                    idxf = pwk.tile([128, K], F32, tag="idxf")
                    nc.vector.tensor_copy(idxf[:],
                                          goacc[:, tt, 2:2+K].bitcast(U32))
                    gates = pwk.tile([128, K], F32, tag="gts")
                    nc.vector.tensor_copy(gates[:], top8[:, 0:K])
                    for e in range(E):
                        if float(bias_vals[e]) == 0.0:
                            continue
                        msk = pwk.tile([128, K], F32, tag="msk")
                        nc.vector.tensor_scalar(msk[:], idxf[:], float(e),
                                                None, op0=ALU.is_equal)
                        nc.vector.tensor_scalar_mul(msk[:], msk[:],
                                                    -float(bias_vals[e]))
                        nc.vector.tensor_add(gates[:], gates[:], msk[:])
                    nc.vector.tensor_copy(goacc[:, tt, 0:K], gates[:])
                if tt == NT // 2 - 1:
                    nc.sync.dma_start(go_o[:, 0:NT//2], goacc[:, 0:NT//2])
            nc.sync.dma_start(go_o[:, NT//2:NT], goacc[:, NT//2:NT])
    nc.compile()
    return nc


# -------------------------------------------------------------- B: experts
def _b_widths(cape):
    widths = []
    rem = cape
    for w in (256, 256, 256):
        if rem <= 0:
            break
        w = min(w, rem)
        widths.append(w)
        rem -= w
    while rem > 768:
        widths.append(512)
        rem -= 512
    if rem > 512:
        a = -(-rem // 64) * 32
        widths += [a, rem - a]
    elif rem:
        widths.append(rem)
    return widths


def build_B(cape):
    assert cape % 2 == 0
    widths = _b_widths(cape)
    ntiles = len(widths)
    starts = [sum(widths[:i]) for i in range(ntiles)]
    ngated = min(3, ntiles)     # leading tiles use output-side gating

    nc = bacc.Bacc("TRN2", target_bir_lowering=False, debug=False,
                   num_devices=NCORES)
    # per-tile contiguous blocks: tile i at [128, 8*starts[i] : +8*widths[i]],
    # inside a block value (p, k, t) = x16[token starts[i]+t, k*128+p]
    # f16 x is pre-scaled *16 and f16 w1/w3 *256 on the host (exact
    # exponent shifts) so the f16 and fp8-DoubleRow matmul terms share one
    # 4096x PSUM scale; gt12 = gate*2^-12 folds the descale into the
    # output-side gate multiply before silu / the glu mul
    xeP = nc.dram_tensor("xeP", [128, 8 * cape], F16, kind="ExternalInput").ap()
    gatb = nc.dram_tensor("gatb", [128, cape], F16, kind="ExternalInput").ap()
    gt12b = nc.dram_tensor("gt12b", [128, cape], F16, kind="ExternalInput").ap()
    w1M = nc.dram_tensor("w1M", [8, 128, 8, 128], F16, kind="ExternalInput").ap()
    w1F8 = nc.dram_tensor("w1F8", [128, 2, H], F8, kind="ExternalInput").ap()
    w3T = nc.dram_tensor("w3T", [D, H], F16, kind="ExternalInput").ap()
    w2T = nc.dram_tensor("w2T", [H, D], F16, kind="ExternalInput").ap()
    yT_o = nc.dram_tensor("yT", [D, cape], F16, kind="ExternalOutput").ap()

    with tile.TileContext(nc) as tc:
        with tc.tile_pool(name="pin", bufs=1) as pin, \
             tc.tile_pool(name="pps", bufs=2, space="PSUM") as pps, \
             tc.tile_pool(name="px", bufs=4) as px, \
             tc.tile_pool(name="px8", bufs=4) as px8, \
             tc.tile_pool(name="pg", bufs=4) as pg, \
             tc.tile_pool(name="pmg", bufs=3) as pmg, \
             tc.tile_pool(name="ps1", bufs=4) as ps1, \
             tc.tile_pool(name="po", bufs=2) as po:
            _warmup(nc, pin, pps, n=7, tag="y", bufs=2)
            gat_sb = pin.tile([128, cape], F16)
            gt12_sb = pin.tile([128, cape], F16)
            w1r = pin.tile([128, 8, 8, 128], F16)   # [p, m, k, c]
            w1q = pin.tile([128, 2, H], F8)         # [p, i, h] fp8 d[0:256]
            w3r = pin.tile([128, 8, H], F16)
            w2r = pin.tile([128, 8, D], F16)

            def load_xs_raw(t):
                tw = widths[t]
                t0 = starts[t]
                xs = px.tile([128, 8, 512], F16, tag="xs")
                src = xeP[:, 8*t0:8*(t0+tw)].rearrange("p (k t) -> p k t", k=8)
                nc.sync.dma_start(xs[:, :, 0:tw], src)
                # device-side fp8 copy of d[0:256] for the DoubleRow part
                # (x already carries the *16 scale)
                x8 = px8.tile([128, 2, 512], F8, tag="x8")
                nc.vector.tensor_copy(x8[:, :, 0:tw], xs[:, 0:2, 0:tw])
                return xs, x8

            # startup stream: w1 m-block 0, fp8 w1 slice, tile-0 x, gate
            # slices, w1 m-blocks 1-7, x tiles 1-2, big weights
            nc.sync.dma_start(w1r[:, 0, :, :], w1M[0])
            nc.sync.dma_start(w1q[:], w1F8[:])
            xs_t = [None] * ntiles
            x8_t = [None] * ntiles
            xs_t[0], x8_t[0] = load_xs_raw(0)
            g01 = sum(widths[0:ngated])
            nc.sync.dma_start(gt12_sb[:, 0:g01], gt12b[:, 0:g01])
            for m in range(1, 8):
                nc.sync.dma_start(w1r[:, m, :, :], w1M[m])
            for t in range(1, ngated):
                xs_t[t], x8_t[t] = load_xs_raw(t)
            nc.sync.dma_start(w3r[:], w3T.rearrange("(k p) h -> p k h", p=128))
            if ntiles > ngated:
                xs_t[ngated], x8_t[ngated] = load_xs_raw(ngated)
            nc.sync.dma_start(gat_sb[:], gatb[:])
            nc.sync.dma_start(w2r[:], w2T.rearrange("(m p) d -> p m d", p=128))
            if ntiles > ngated:
                nc.sync.dma_start(gt12_sb[:, g01:cape], gt12b[:, g01:cape])

            def h1_phase(t):
                xs = xs_t[t]
                x8 = x8_t[t]
                tw = widths[t]
                t0 = starts[t]
                nhs = [(0, min(256, tw))] + ([(256, tw)] if tw > 256 else [])
                s1a = ps1.tile([128, 8, 512], F16, tag="s1a")
                for m in range(8):
                    h1 = pps.tile([128, 512], F32, tag="h1")
                    # f16 k=2 opens the psum group over the full [128, tw]
                    # zero-region; f16 k=7 closes it
                    nc.tensor.matmul(h1[:, 0:tw], w1r[:, m, 2, :],
                                     xs[:, 2, 0:tw], start=True, stop=False)
                    # d[0:256] in fp8 DoubleRow. Lower 64 output rows
                    # accumulate straight into h1 (mixed group, base 0);
                    # the backend rejects DR at base 64, so the upper rows
                    # land in their own [64, n] psum tiles and merge below.
                    # one h8 tile per m-block: both n-halves live as
                    # column-disjoint psum groups in one bank, so the ring
                    # advances per-m (not per-half) and the next m-block's
                    # DR doesn't wait on this one's DVE drain
                    h8 = pps.tile([64, 512], F32, tag="h8", bufs=2)
                    for (n0, n1) in nhs:
                        nc.tensor.matmul(h1[0:64, n0:n1],
                                         w1q[:, :, m*128:m*128+64],
                                         x8[:, :, n0:n1],
                                         start=False, stop=False, perf_mode=DR)
                        nc.tensor.matmul(h8[:, n0:n1],
                                         w1q[:, :, m*128+64:m*128+128],
                                         x8[:, :, n0:n1],
                                         start=True, stop=True, perf_mode=DR)
                    for k in range(3, 8):
                        nc.tensor.matmul(h1[:, 0:tw], w1r[:, m, k, :],
                                         xs[:, k, 0:tw],
                                         start=False, stop=(k == 7))
                    # descale the upper-row fp8 parts early (overlaps the
                    # f16 matmuls), then gate-descale h1 and merge with a
                    # cross-partition add
                    # the descale mul carries the partition shift (out at
                    # base 64, both inputs at base 0 — a 2-input-base op,
                    # which walrus accepts; a 3-base add does not)
                    tmps = []
                    for (n0, n1) in nhs:
                        tmp = pmg.tile([128, 256], F16, tag="tmp")
                        nc.vector.tensor_mul(tmp[64:128, 0:n1-n0],
                                             h8[:, n0:n1],
                                             gt12_sb[0:64, t0+n0:t0+n1])
                        tmps.append((tmp, n0, n1))
                    m1 = pmg.tile([128, 512], F16, tag="m1")
                    nc.vector.tensor_mul(m1[:, 0:tw], h1[:, 0:tw],
                                         gt12_sb[:, t0:t0+tw])
                    for (tmp, n0, n1) in tmps:
                        nc.vector.tensor_add(m1[64:128, n0:n1],
                                             m1[64:128, n0:n1],
                                             tmp[64:128, 0:n1-n0])
                    nc.scalar.activation(s1a[:, m, 0:tw], m1[:, 0:tw],
                                         AF.Silu)
                return s1a

            def h3_phase(t, s1a):
                xs = xs_t[t]
                tw = widths[t]
                t0 = starts[t]
                gT = pg.tile([128, 8, 512], F16, tag="gT")
                for m in range(8):
                    h3 = pps.tile([128, 512], F32, tag="h3")
                    for k in range(8):
                        nc.tensor.matmul(h3[:, 0:tw], w3r[:, k, m*128:(m+1)*128],
                                         xs[:, k, 0:tw],
                                         start=(k == 0), stop=(k == 7))
                    m3 = pmg.tile([128, 512], F16, tag="m3")
                    nc.vector.tensor_mul(m3[:, 0:tw], h3[:, 0:tw],
                                         gt12_sb[:, t0:t0+tw])
                    nc.vector.tensor_mul(gT[:, m, 0:tw], s1a[:, m, 0:tw],
                                         m3[:, 0:tw])
                return gT

            def down(gT, t):
                tw = widths[t]
                t0 = starts[t]
                last = (t == ntiles - 1)
                osb = po.tile([128, 8, 512], F16, tag="osb")
                yTr = yT_o.rearrange("(d p) t -> p d t", p=128)
                for d in range(8):
                    yp = pps.tile([128, 512], F32, tag="y")
                    for m in range(8):
                        nc.tensor.matmul(yp[:, 0:tw], w2r[:, m, d*128:(d+1)*128],
                                         gT[:, m, 0:tw],
                                         start=(m == 0), stop=(m == 7))
                    # PSUM->SBUF copy fused with the output-side routing scale
                    nc.vector.tensor_mul(osb[:, d, 0:tw], yp[:, 0:tw],
                                         gat_sb[:, t0:t0+tw])
                    if last:
                        nc.sync.dma_start(yTr[:, d, t0:t0+tw], osb[:, d, 0:tw])
                if not last:
                    nc.gpsimd.dma_start(yTr[:, :, t0:t0+tw], osb[:, :, 0:tw])

            # pipeline: ngated h1 phases up front (w1-only work) so the PE
            # has runway while w3 streams; then rotate h1/h3/down
            s1a_t = [None] * ntiles
            gT_t = [None] * ntiles
            for t in range(ngated):
                s1a_t[t] = h1_phase(t)
            for t in range(ngated):
                gT_t[t] = h3_phase(t, s1a_t[t])
            if ntiles > ngated:
                for j in range(max(1, ngated - 2)):
                    down(gT_t[j], j)
                for t in range(ngated, ntiles):
                    if xs_t[t] is None:
                        xs_t[t], x8_t[t] = load_xs_raw(t)
                    s1a_t[t] = h1_phase(t)
                    gT_t[t] = h3_phase(t, s1a_t[t])
                    down(gT_t[t - 2], t - 2)
                down(gT_t[ntiles - 2], ntiles - 2)
                down(gT_t[ntiles - 1], ntiles - 1)
            else:
                for t in range(ntiles):
                    down(gT_t[t], t)
    nc.compile()
    return nc


# ------------------------------------------------------ L3: shared + combine
def build_l3():
    nc = bacc.Bacc("TRN2", target_bir_lowering=False, debug=False,
                   num_devices=NCORES)
    xP = nc.dram_tensor("xP", [128, 8, 8, 128], F16,
                        kind="ExternalInput").ap()
    sw1B = nc.dram_tensor("sw1B", [8, 128, 8, 128], F16,
                          kind="ExternalInput").ap()
    sw3B = nc.dram_tensor("sw3B", [D, H], F16, kind="ExternalInput").ap()
    sw2B = nc.dram_tensor("sw2B", [H, D], F16, kind="ExternalInput").ap()
    AT = nc.dram_tensor("AT", [D, TPC], F16, kind="ExternalInput").ap()
    BT = nc.dram_tensor("BT", [D, TPC], F16, kind="ExternalInput").ap()
    out_o = nc.dram_tensor("outT", [D, TPC], F16, kind="ExternalOutput").ap()
    # phases over 128-token chunks: 4 quarters of 256 (phase width below
    # 256 makes the h1 m-cadence outrun the w1 DMA stream and stalls)
    CH = [(0, 2), (2, 4), (4, 6), (6, 8)]
    NM = 8

    with tile.TileContext(nc) as tc:
        with tc.tile_pool(name="pin", bufs=1) as pin, \
             tc.tile_pool(name="pps", bufs=2, space="PSUM") as pps, \
             tc.tile_pool(name="pg", bufs=3) as pg, \
             tc.tile_pool(name="ps1", bufs=3) as ps1, \
             tc.tile_pool(name="po", bufs=2) as po:
            _warmup(nc, pin, pps, n=7)
            xs = pin.tile([128, 8, 8, 128], F16)
            w1r = pin.tile([128, 8, 8, 128], F16)
            w3r = pin.tile([128, 8, H], F16)
            w2r = pin.tile([128, 8, D], F16)
            absum = pin.tile([128, 8, TPC], F16)
            bt_sb = pin.tile([128, 8, TPC], F16)
            ATr = AT.rearrange("(d p) t -> p d t", p=128)
            BTr = BT.rearrange("(d p) t -> p d t", p=128)

            nc.sync.dma_start(w1r[:, 0], sw1B[0])
            nc.sync.dma_start(xs[:, 0], xP[:, 0])
            nc.sync.dma_start(xs[:, 1], xP[:, 1])
            for m in range(1, 8):
                nc.sync.dma_start(w1r[:, m], sw1B[m])
            nc.sync.dma_start(xs[:, 2:4], xP[:, 2:4])
            nc.sync.dma_start(w3r[:], sw3B.rearrange("(k p) h -> p k h", p=128))
            nc.sync.dma_start(xs[:, 4:6], xP[:, 4:6])
            nc.sync.dma_start(xs[:, 6:8], xP[:, 6:8])
            # combine terms: first halves, then w2, then the rest
            nc.sync.dma_start(absum[:, :, 0:512], ATr[:, :, 0:512])
            nc.sync.dma_start(bt_sb[:, :, 0:512], BTr[:, :, 0:512])
            nc.sync.dma_start(w2r[:], sw2B.rearrange("(m p) d -> p m d", p=128))
            nc.sync.dma_start(absum[:, :, 512:1024], ATr[:, :, 512:1024])
            nc.sync.dma_start(bt_sb[:, :, 512:1024], BTr[:, :, 512:1024])

            def absum_add(half):
                ts = slice(half*512, (half+1)*512)
                nc.vector.tensor_add(absum[:, :, ts], absum[:, :, ts],
                                     bt_sb[:, :, ts])

            def h1_phase(q):
                c0, c1 = CH[q]
                w = (c1 - c0) * 128
                s1a = ps1.tile([128, NM, 256], F16, tag="s1a")
                for m in range(NM):
                    h1 = pps.tile([128, 256], F32, tag="h1")
                    for k in range(8):
                        nc.tensor.matmul(h1[:, 0:w], w1r[:, m, k, :],
                                         xs[:, c0:c1, k, :],
                                         start=(k == 0), stop=(k == 7))
                    nc.scalar.activation(s1a[:, m, 0:w], h1[:, 0:w], AF.Silu)
                return s1a

            def h3_phase(q, s1a):
                c0, c1 = CH[q]
                w = (c1 - c0) * 128
                gT = pg.tile([128, NM, 256], F16, tag="gT")
                for m in range(NM):
                    h3 = pps.tile([128, 256], F32, tag="h3")
                    for k in range(8):
                        nc.tensor.matmul(h3[:, 0:w], w3r[:, k, m*128:(m+1)*128],
                                         xs[:, c0:c1, k, :],
                                         start=(k == 0), stop=(k == 7))
                    nc.vector.tensor_mul(gT[:, m, 0:w], s1a[:, m, 0:w],
                                         h3[:, 0:w])
                return gT

            def down(gT, q):
                c0, c1 = CH[q]
                w = (c1 - c0) * 128
                t0 = c0 * 128
                osb = po.tile([128, 8, 256], F16, tag="osb")
                outr = out_o.rearrange("(d p) t -> p d t", p=128)
                for d in range(8):
                    yp = pps.tile([128, 256], F32, tag="y")
                    for m in range(NM):
                        nc.tensor.matmul(yp[:, 0:w], w2r[:, m, d*128:(d+1)*128],
                                         gT[:, m, 0:w],
                                         start=(m == 0), stop=(m == NM-1))
                    nc.vector.tensor_add(osb[:, d, 0:w], yp[:, 0:w],
                                         absum[:, d, t0:t0+w])
                    nc.sync.dma_start(outr[:, d, t0:t0+w], osb[:, d, 0:w])

            s0 = h1_phase(0)
            s1 = h1_phase(1)
            g0 = h3_phase(0, s0)
            absum_add(0)
            s2 = h1_phase(2)
            g1 = h3_phase(1, s1)
            down(g0, 0)
            s3 = h1_phase(3)
            absum_add(1)
            g2 = h3_phase(2, s2)
            down(g1, 1)
            g3 = h3_phase(3, s3)
            down(g2, 2)
            down(g3, 3)
    nc.compile()
    return nc


_BUILT = {}
_LAST_KEYS = []


def _get(name, builder, *args):
    key = (name,) + tuple(args)
    if key not in _BUILT:
        _BUILT[key] = builder(*args)
    return _BUILT[key], key


def _packx(a, chunk):
    """[Tn, D] -> [128, Tn//chunk, 8, chunk]: out[p, c, k, t] =
    a[c*chunk + t, k*128 + p] (per-chunk-contiguous DMA layout)."""
    n = a.shape[0] // chunk
    return np.ascontiguousarray(
        a.reshape(n, chunk, 8, 128).transpose(3, 0, 2, 1))


def _mblocks(wT16, nm=8):
    # [D, nm*128] -> [m, p, k, c] with [m,p,k,c] = wT[k*128+p, m*128+c]
    return np.ascontiguousarray(
        wT16.reshape(8, 128, nm, 128).transpose(2, 1, 0, 3))


def kernel(**inputs):
    x = np.ascontiguousarray(np.asarray(inputs["x"], dtype=np.float32))
    xf = x.reshape(T, D)
    gw = np.asarray(inputs["gate_w"], dtype=np.float32)
    bias = np.asarray(inputs["expert_bias"], dtype=np.float32)
    w1 = np.asarray(inputs["w1"], dtype=np.float32)
    w2 = np.asarray(inputs["w2"], dtype=np.float32)
    w3 = np.asarray(inputs["w3"], dtype=np.float32)
    sw1 = np.asarray(inputs["sw1"], dtype=np.float32)
    sw2 = np.asarray(inputs["sw2"], dtype=np.float32)
    sw3 = np.asarray(inputs["sw3"], dtype=np.float32)

    cores = list(range(NCORES))
    del _LAST_KEYS[:]

    # ---- L1: router ----
    ncA, kA = _get("l1", build_l1, tuple(float(b) for b in bias))
    _LAST_KEYS.append(kA)
    xf16 = xf.astype(np.float16)
    gw16T = np.ascontiguousarray(gw.astype(np.float16).T)
    rgT = np.ascontiguousarray(
        ((gw.T - gw16T.astype(np.float32)) * 4096.0).astype(np.float16))

    def _prow(a):
        # [D, E] -> per-partition [128, 8*E] rows matching the [p][k][e] reads
        return np.ascontiguousarray(
            a.reshape(8, 128, E).transpose(1, 0, 2)).reshape(128, 8 * E)

    gw16s = (gw16T.astype(np.float32) * 4096.0).astype(np.float16)
    pk = np.zeros((128, 320), np.uint8)
    pk[:, 0:128] = _prow(gw16s).view(np.uint8)
    pk[:, 128:256] = _prow(rgT).view(np.uint8)
    gwpk = np.ascontiguousarray(pk).view(np.float32)
    xPl = [_packx(xf16[c*TPC:(c+1)*TPC], 128) for c in cores]
    inA = [{"x16P": xPl[c], "gwpk": gwpk} for c in cores]
    rA = run_bass_kernel_spmd(ncA, inA, cores).results
    gates_l, sel_l = [], []
    for r in rA:
        go = np.ascontiguousarray(r["go"])     # [128, NT, 10]
        gates_l.append(go[:, :, 0:2].transpose(1, 0, 2).reshape(TPC, K))
        sel_l.append(
            go.view(np.uint32)[:, :, 2:4].transpose(1, 0, 2).reshape(TPC, K))
    gates = np.concatenate(gates_l)
    sel = np.concatenate(sel_l)

    # ---- host dispatch (pure data movement: stable sort by expert) ----
    flat = sel.reshape(-1).astype(np.int64)
    order = np.argsort(flat, kind="stable")               # [T*K]
    toks = order // K
    kslot = order % K
    gs = gates.reshape(-1)[order]
    counts = np.bincount(flat, minlength=E)
    bounds = np.concatenate([[0], np.cumsum(counts)])
    cape = int(-(-int(counts.max()) // 2) * 2)

    # ---- B: experts ----
    ncB, kB = _get("B", build_B, cape)
    _LAST_KEYS.append(kB)
    widths = _b_widths(cape)
    starts = [sum(widths[:i]) for i in range(len(widths))]
    inB = []
    for e in cores:
        n = int(counts[e])
        sl = slice(int(bounds[e]), int(bounds[e+1]))
        # f16 x pre-scaled *16 so the f16 and fp8 matmul terms share one
        # 4096x psum scale (w1/w3 carry *256)
        xe = np.zeros((cape, D), np.float16)
        xe[:n] = (xf16[toks[sl]].astype(np.float32) * 16.0).astype(np.float16)
        xeP = np.concatenate(
            [_packx(xe[t0:t0+tw], tw).reshape(128, 8*tw)
             for t0, tw in zip(starts, widths)], axis=1)
        gb = np.zeros((128, cape), np.float16)
        gb[:, :n] = gs[sl].astype(np.float16)[None, :]
        gt12 = (gb.astype(np.float32) * (1.0 / 4096.0)).astype(np.float16)
        w1t = w1[e].T.astype(np.float32) * 256.0  # [D, H]
        w3t = w3[e].T.astype(np.float32) * 256.0
        # fp8 stationary [p, i, h] = w1[h, i*128+p]*256
        w1f8 = np.ascontiguousarray(
            w1t[0:256].reshape(2, 128, H).transpose(1, 0, 2)).astype(F8NP)
        inB.append({
            "xeP": np.ascontiguousarray(xeP),
            "gatb": gb,
            "gt12b": gt12,
            "w1M": _mblocks(w1t.astype(np.float16)),
            "w1F8": w1f8,
            "w3T": np.ascontiguousarray(w3t.astype(np.float16)),
            "w2T": np.ascontiguousarray(w2[e].T).astype(np.float16),
        })
    rB = run_bass_kernel_spmd(ncB, inB, cores).results

    # ---- host combine prep (pure data movement: permutation) ----
    A = np.zeros((T, D), np.float16)
    B = np.zeros((T, D), np.float16)
    for e in cores:
        n = int(counts[e])
        sl = slice(int(bounds[e]), int(bounds[e+1]))
        rows = rB[e]["yT"][:, :n].T                       # [n, D] f16
        tsel = toks[sl]
        ksel = kslot[sl]
        A[tsel[ksel == 0]] = rows[ksel == 0]
        B[tsel[ksel == 1]] = rows[ksel == 1]

    # ---- L3: shared + combine ----
    ncC, kC = _get("l3", build_l3)
    _LAST_KEYS.append(kC)
    sw1B = _mblocks(sw1.T.astype(np.float16))
    sw3B = np.ascontiguousarray(sw3.T).astype(np.float16)
    sw2B = np.ascontiguousarray(sw2.T).astype(np.float16)
    inC = []
    for c in cores:
        sl = slice(c*TPC, (c+1)*TPC)
        inC.append({
            "xP": xPl[c],
            "sw1B": sw1B, "sw3B": sw3B, "sw2B": sw2B,
            "AT": np.ascontiguousarray(A[sl].T),
            "BT": np.ascontiguousarray(B[sl].T),
        })
    rC = run_bass_kernel_spmd(ncC, inC, cores).results
    out = np.concatenate([r["outT"].astype(np.float32).T for r in rC])
    return out.reshape(x.shape).astype(inputs["x"].dtype, copy=False)
